# revision 20
# baseline (speedup 1.0000x reference)
"""GumbelSelector Trainium2 kernel.

Math: h = relu(s @ W1 + b1); lo = h @ W2 + b2  (2 classes)
  dec  = (argmax(lo) == 1)  ==  (z > 0)         where z = h @ (W2[:,1]-W2[:,0]) + (b2[1]-b2[0])
  prob = softmax(lo)[..., 1] ==  sigmoid(z)
  Per-row correction (LB=1): if a row of dec is all zero, activate argmax(rnoise).

Sharding: data-parallel over batch B=64 -> 8 cores x 8 rows. Weights replicated.

Device computes z for every token with fp16 operands (f32 PSUM accumulation);
fp16 keeps the matmul at 1 cycle/row (fp32 is 4) and halves HBM traffic. Host
pre-transposes each core's s shard to [D=256, 32768] fp16 so DMA loads are
coalesced with the contraction dim on SBUF partitions, then finishes
elementwise: prob = sigmoid(z), dec = z > 0, an exact f64 recompute of the
~0.5% of tokens with |z| < tau (fp16 max z error is ~1.6e-3, tau = 5e-3), and
the LB row correction.

The z matmuls have M=1, so each group of 4 tiles lands in one PSUM bank at
column positions 0/32/64/96 (distinct PE column groups run concurrently) and
a single vector copy moves all 4 results to SBUF; z drains to DRAM in 8 KiB
strided DMAs whose permutation the host undoes for free.
"""

import sys

if "/opt/trn_rl_repo" not in sys.path:
    sys.path.insert(0, "/opt/trn_rl_repo")

import numpy as np

import concourse.bass as bass
import concourse.mybir as mybir
import concourse.tile as tile
from concourse import bacc
from concourse.bass_utils import run_bass_kernel_spmd

B, N, D = 64, 4096, 256
HID = D // 2  # 128
NCORES = 8
BPC = B // NCORES          # batch rows per core
TOK = BPC * N              # 32768 tokens per core
TS = 512                   # tokens per compute tile (1 PSUM bank)
GROUP = 4                  # tiles per z-group (PE column groups 0/32/64/96)
# staged slab sizes: small first slabs let compute start early, then 8 KiB
# per-partition loads for bandwidth
SLAB_PLAN = [512, 512, 1024, 2048] + [4096] * 7
TAU = 5e-3                 # |z| window for exact host recompute
F32 = mybir.dt.float32
F16 = mybir.dt.float16

_NC = None


def _build_nc():
    nc = bacc.Bacc("TRN2", target_bir_lowering=False, debug=False)
    sT = nc.dram_tensor("sT", [D, TOK], F16, kind="ExternalInput")
    w1 = nc.dram_tensor("w1", [D, HID], F16, kind="ExternalInput")
    b1 = nc.dram_tensor("b1", [HID, 1], F32, kind="ExternalInput")
    w2 = nc.dram_tensor("w2", [HID, 1], F16, kind="ExternalInput")
    zout = nc.dram_tensor("zout", [1, TOK], F16, kind="ExternalOutput")

    AF = mybir.ActivationFunctionType

    with tile.TileContext(nc) as tc:
        with (
            tc.tile_pool(name="consts", bufs=1) as consts,
            tc.tile_pool(name="sload", bufs=5) as sload,
            tc.tile_pool(name="hpool", bufs=8) as hpool,
            tc.tile_pool(name="zgpool", bufs=3) as zgpool,
            tc.tile_pool(name="phpool", bufs=4, space=bass.MemorySpace.PSUM) as phpool,
            tc.tile_pool(name="pzpool", bufs=2, space=bass.MemorySpace.PSUM) as pzpool,
            tc.tile_pool(name="pwpool", bufs=1, space=bass.MemorySpace.PSUM) as pwpool,
        ):
            w1a = consts.tile([128, HID], F16)
            nc.gpsimd.dma_start(w1a[:], w1[0:128, :])
            w1b = consts.tile([128, HID], F16)
            nc.gpsimd.dma_start(w1b[:], w1[128:256, :])
            b1s = consts.tile([HID, 1], F32)
            nc.gpsimd.dma_start(b1s[:], b1[:])
            w2s = consts.tile([HID, 1], F16)
            nc.gpsimd.dma_start(w2s[:], w2[:])

            # ~3.5us of garbage matmuls at startup: keeps the PE busy through
            # one HAM activity window so the clock gate opens (1.2 -> 2.4
            # GHz) before the first real tile; overlaps the slab-0 DMA.
            warm = consts.tile([128, TS], F16)
            nc.vector.memset(warm[:], 0.0)
            pwarm = pwpool.tile([128, TS], F32)
            for _ in range(18):
                nc.tensor.matmul(pwarm[:], w1a[:], warm[:], start=True, stop=True)

            state = {"zq": [], "g": 0, "zgb": None}
            ALU = mybir.AluOpType

            def flush_zgroup():
                zgb = zgpool.tile([97, TS], F16, tag="zgb")
                pzg = pzpool.tile([128, TS], F32)
                for slot, hp in enumerate(state["zq"][:GROUP]):
                    nc.tensor.matmul(pzg[32 * slot : 32 * slot + 1, :], w2s[:],
                                     hp[:], start=True, stop=True,
                                     tile_position=(0, 32 * slot))
                nc.vector.tensor_scalar_add(zgb[0:97, :], pzg[0:97, :], 0.0)
                # strided drain rows 0/32/64/96 land in token order, so the
                # host needs no permutation; scalar ring keeps sync free
                goff = state["g"] * GROUP * TS
                nc.scalar.dma_start(zout[0:1, goff : goff + GROUP * TS],
                                    zgb[0:97:32, :])
                state["g"] += 1
                state["zq"] = state["zq"][GROUP:]

            off = 0
            ti = 0
            for slab in SLAB_PLAN:
                sa = sload.tile([128, slab], F16, tag="sa")
                sb = sload.tile([128, slab], F16, tag="sb")
                # both on the sync HWDGE ring: the sync engine is idle so
                # ring pushes are never delayed behind compute work
                nc.sync.dma_start(sa[:], sT[0:128, off : off + slab])
                nc.sync.dma_start(sb[:], sT[128:256, off : off + slab])
                for c in range(slab // TS):
                    hoff = c * TS
                    ph = phpool.tile([128, TS], F32)
                    nc.tensor.matmul(ph[:], w1a[:], sa[:, hoff : hoff + TS],
                                     start=True, stop=False)
                    nc.tensor.matmul(ph[:], w1b[:], sb[:, hoff : hoff + TS],
                                     start=False, stop=True)
                    # flush with 2 tiles of lag so the group's relus (paced
                    # by scalar/vector throughput) are done before the PE's
                    # in-order queue reaches the 4 concurrent z matmuls
                    if len(state["zq"]) >= GROUP + 2:
                        flush_zgroup()
                    h = hpool.tile([128, TS], F16)
                    # alternate relu between scalar and vector so neither
                    # engine paces the pipeline
                    if ti % 2 == 0:
                        nc.scalar.activation(h[:], ph[:], AF.Relu, bias=b1s[:])
                    else:
                        nc.vector.tensor_scalar(h[:], ph[:], b1s[:], 0.0,
                                                ALU.add, ALU.max)
                    state["zq"].append(h)
                    ti += 1
                off += slab
            while state["zq"]:
                flush_zgroup()

    nc.compile()
    return nc


def _get_nc():
    global _NC
    if _NC is None:
        _NC = _build_nc()
    return _NC


def _make_in_maps(s, W1, b1, W2, b2, rnoise):
    s16 = np.asarray(s, dtype=np.float16)
    # [NCORES, D, TOK] with the contraction dim outer -> coalesced loads
    sT = np.ascontiguousarray(s16.reshape(NCORES, TOK, D).transpose(0, 2, 1))
    w1h = np.ascontiguousarray(W1, dtype=np.float16)
    b1c = np.ascontiguousarray(b1, dtype=np.float32).reshape(HID, 1)
    w2h = np.ascontiguousarray(W2[:, 1] - W2[:, 0], dtype=np.float16).reshape(HID, 1)
    return [
        {"sT": sT[c], "w1": w1h, "b1": b1c, "w2": w2h}
        for c in range(NCORES)
    ]


def run(s, W1, b1, W2, b2, rnoise, trace=False):
    nc = _get_nc()
    in_maps = _make_in_maps(s, W1, b1, W2, b2, rnoise)
    res = run_bass_kernel_spmd(nc, in_maps, list(range(NCORES)), trace=trace)
    b2d = np.float32(b2[1] - b2[0])
    z = np.stack([r["zout"].reshape(TOK) for r in res.results])
    z = z.reshape(B, N).astype(np.float32) + b2d

    dec = z > 0
    prob = 1.0 / (1.0 + np.exp(-z.astype(np.float64)))

    # Exact recompute of borderline tokens (fp16 z error < 1.6e-3 << TAU).
    bi, ni = np.nonzero(np.abs(z) < TAU)
    if bi.size:
        sv = np.asarray(s, dtype=np.float64)[bi, ni]
        hv = np.maximum(sv @ np.asarray(W1, np.float64) + np.asarray(b1, np.float64), 0)
        zv = hv @ np.asarray(W2[:, 1] - W2[:, 0], np.float64) + float(b2d)
        dec[bi, ni] = zv > 0
        prob[bi, ni] = 1.0 / (1.0 + np.exp(-zv))

    dec = dec.astype(np.float32)
    # LB=1 row correction: a row with no active slot activates argmax(rnoise)
    rn = np.asarray(rnoise)
    for b in np.nonzero(dec.sum(axis=1) == 0)[0]:
        dec[b, np.argmax(rn[b])] = 1.0

    return (dec, prob.astype(np.float32)), res


def kernel(s, W1, b1, W2, b2, rnoise):
    (dec, prob), _ = run(s, W1, b1, W2, b2, rnoise)
    return dec, prob


# revision 21
# speedup vs baseline: 1.0395x; 1.0395x over previous
"""GumbelSelector Trainium2 kernel.

Math: h = relu(s @ W1 + b1); lo = h @ W2 + b2  (2 classes)
  dec  = (argmax(lo) == 1)  ==  (z > 0)         where z = h @ (W2[:,1]-W2[:,0]) + (b2[1]-b2[0])
  prob = softmax(lo)[..., 1] ==  sigmoid(z)
  Per-row correction (LB=1): if a row of dec is all zero, activate argmax(rnoise).

Sharding: data-parallel over batch B=64 -> 8 cores x 8 rows. Weights replicated.

Device computes z for every token with fp16 operands (f32 PSUM accumulation);
fp16 keeps the matmul at 1 cycle/row (fp32 is 4) and halves HBM traffic. Host
pre-transposes each core's s shard to [D=256, 32768] fp16 so DMA loads are
coalesced with the contraction dim on SBUF partitions, then finishes
elementwise: prob = sigmoid(z), dec = z > 0, an exact f64 recompute of the
~0.5% of tokens with |z| < tau (fp16 max z error is ~1.6e-3, tau = 5e-3), and
the LB row correction.

The z matmuls have M=1, so each group of 4 tiles lands in one PSUM bank at
column positions 0/32/64/96 (distinct PE column groups run concurrently) and
a single vector copy moves all 4 results to SBUF; z drains to DRAM in 8 KiB
strided DMAs whose permutation the host undoes for free.
"""

import sys

if "/opt/trn_rl_repo" not in sys.path:
    sys.path.insert(0, "/opt/trn_rl_repo")

import numpy as np

import concourse.bass as bass
import concourse.mybir as mybir
import concourse.tile as tile
from concourse import bacc
from concourse.bass_utils import run_bass_kernel_spmd

B, N, D = 64, 4096, 256
HID = D // 2  # 128
NCORES = 8
BPC = B // NCORES          # batch rows per core
TOK = BPC * N              # 32768 tokens per core
TS = 512                   # tokens per compute tile (1 PSUM bank)
GROUP = 4                  # tiles per z-group (PE column groups 0/32/64/96)
# staged slab sizes: small first slabs let compute start early, then 8 KiB
# per-partition loads for bandwidth
SLAB_PLAN = [512, 512, 1024, 2048] + [4096] * 7
TAU = 5e-3                 # |z| window for exact host recompute
F32 = mybir.dt.float32
F16 = mybir.dt.float16

_NC = None


def _build_nc():
    nc = bacc.Bacc("TRN2", target_bir_lowering=False, debug=False)
    sT = nc.dram_tensor("sT", [D, TOK], F16, kind="ExternalInput")
    w1 = nc.dram_tensor("w1", [D, HID], F16, kind="ExternalInput")
    b1 = nc.dram_tensor("b1", [HID, 1], F32, kind="ExternalInput")
    w2 = nc.dram_tensor("w2", [HID, 1], F16, kind="ExternalInput")
    zout = nc.dram_tensor("zout", [1, TOK], F16, kind="ExternalOutput")

    AF = mybir.ActivationFunctionType

    with tile.TileContext(nc) as tc:
        with (
            tc.tile_pool(name="consts", bufs=1) as consts,
            tc.tile_pool(name="sload", bufs=5) as sload,
            tc.tile_pool(name="hpool", bufs=8) as hpool,
            tc.tile_pool(name="zgpool", bufs=3) as zgpool,
            tc.tile_pool(name="phpool", bufs=4, space=bass.MemorySpace.PSUM) as phpool,
            tc.tile_pool(name="pzpool", bufs=2, space=bass.MemorySpace.PSUM) as pzpool,
            tc.tile_pool(name="pwpool", bufs=1, space=bass.MemorySpace.PSUM) as pwpool,
        ):
            w1a = consts.tile([128, HID], F16)
            nc.gpsimd.dma_start(w1a[:], w1[0:128, :])
            w1b = consts.tile([128, HID], F16)
            nc.gpsimd.dma_start(w1b[:], w1[128:256, :])
            b1s = consts.tile([HID, 1], F32)
            nc.gpsimd.dma_start(b1s[:], b1[:])
            w2s = consts.tile([HID, 1], F16)
            nc.gpsimd.dma_start(w2s[:], w2[:])

            # ~3.5us of garbage matmuls at startup: keeps the PE busy through
            # one HAM activity window so the clock gate opens (1.2 -> 2.4
            # GHz) before the first real tile; overlaps the slab-0 DMA.
            warm = consts.tile([128, TS], F16)
            nc.vector.memset(warm[:], 0.0)
            pwarm = pwpool.tile([128, TS], F32)
            for _ in range(15):
                nc.tensor.matmul(pwarm[:], w1a[:], warm[:], start=True, stop=True)

            state = {"zq": [], "g": 0, "zgb": None}
            ALU = mybir.AluOpType

            def flush_zgroup():
                zgb = zgpool.tile([97, TS], F16, tag="zgb")
                pzg = pzpool.tile([128, TS], F32)
                for slot, hp in enumerate(state["zq"][:GROUP]):
                    nc.tensor.matmul(pzg[32 * slot : 32 * slot + 1, :], w2s[:],
                                     hp[:], start=True, stop=True,
                                     tile_position=(0, 32 * slot))
                nc.vector.tensor_scalar_add(zgb[0:97, :], pzg[0:97, :], 0.0)
                # strided drain rows 0/32/64/96 land in token order, so the
                # host needs no permutation; scalar ring keeps sync free
                goff = state["g"] * GROUP * TS
                nc.scalar.dma_start(zout[0:1, goff : goff + GROUP * TS],
                                    zgb[0:97:32, :])
                state["g"] += 1
                state["zq"] = state["zq"][GROUP:]

            off = 0
            ti = 0
            for slab in SLAB_PLAN:
                sa = sload.tile([128, slab], F16, tag="sa")
                sb = sload.tile([128, slab], F16, tag="sb")
                # both on the sync HWDGE ring: the sync engine is idle so
                # ring pushes are never delayed behind compute work
                nc.sync.dma_start(sa[:], sT[0:128, off : off + slab])
                nc.sync.dma_start(sb[:], sT[128:256, off : off + slab])
                for c in range(slab // TS):
                    hoff = c * TS
                    ph = phpool.tile([128, TS], F32)
                    nc.tensor.matmul(ph[:], w1a[:], sa[:, hoff : hoff + TS],
                                     start=True, stop=False)
                    nc.tensor.matmul(ph[:], w1b[:], sb[:, hoff : hoff + TS],
                                     start=False, stop=True)
                    # flush with 2 tiles of lag so the group's relus (paced
                    # by scalar/vector throughput) are done before the PE's
                    # in-order queue reaches the 4 concurrent z matmuls
                    if len(state["zq"]) >= GROUP + 2:
                        flush_zgroup()
                    h = hpool.tile([128, TS], F16)
                    # alternate relu between scalar and vector so neither
                    # engine paces the pipeline
                    if ti % 2 == 0:
                        nc.scalar.activation(h[:], ph[:], AF.Relu, bias=b1s[:])
                    else:
                        nc.vector.tensor_scalar(h[:], ph[:], b1s[:], 0.0,
                                                ALU.add, ALU.max)
                    state["zq"].append(h)
                    ti += 1
                off += slab
            while state["zq"]:
                flush_zgroup()

    nc.compile()
    return nc


def _get_nc():
    global _NC
    if _NC is None:
        _NC = _build_nc()
    return _NC


def _make_in_maps(s, W1, b1, W2, b2, rnoise):
    s16 = np.asarray(s, dtype=np.float16)
    # [NCORES, D, TOK] with the contraction dim outer -> coalesced loads
    sT = np.ascontiguousarray(s16.reshape(NCORES, TOK, D).transpose(0, 2, 1))
    w1h = np.ascontiguousarray(W1, dtype=np.float16)
    b1c = np.ascontiguousarray(b1, dtype=np.float32).reshape(HID, 1)
    w2h = np.ascontiguousarray(W2[:, 1] - W2[:, 0], dtype=np.float16).reshape(HID, 1)
    return [
        {"sT": sT[c], "w1": w1h, "b1": b1c, "w2": w2h}
        for c in range(NCORES)
    ]


def run(s, W1, b1, W2, b2, rnoise, trace=False):
    nc = _get_nc()
    in_maps = _make_in_maps(s, W1, b1, W2, b2, rnoise)
    res = run_bass_kernel_spmd(nc, in_maps, list(range(NCORES)), trace=trace)
    b2d = np.float32(b2[1] - b2[0])
    z = np.stack([r["zout"].reshape(TOK) for r in res.results])
    z = z.reshape(B, N).astype(np.float32) + b2d

    dec = z > 0
    prob = 1.0 / (1.0 + np.exp(-z.astype(np.float64)))

    # Exact recompute of borderline tokens (fp16 z error < 1.6e-3 << TAU).
    bi, ni = np.nonzero(np.abs(z) < TAU)
    if bi.size:
        sv = np.asarray(s, dtype=np.float64)[bi, ni]
        hv = np.maximum(sv @ np.asarray(W1, np.float64) + np.asarray(b1, np.float64), 0)
        zv = hv @ np.asarray(W2[:, 1] - W2[:, 0], np.float64) + float(b2d)
        dec[bi, ni] = zv > 0
        prob[bi, ni] = 1.0 / (1.0 + np.exp(-zv))

    dec = dec.astype(np.float32)
    # LB=1 row correction: a row with no active slot activates argmax(rnoise)
    rn = np.asarray(rnoise)
    for b in np.nonzero(dec.sum(axis=1) == 0)[0]:
        dec[b, np.argmax(rn[b])] = 1.0

    return (dec, prob.astype(np.float32)), res


def kernel(s, W1, b1, W2, b2, rnoise):
    (dec, prob), _ = run(s, W1, b1, W2, b2, rnoise)
    return dec, prob
